# revision 21
# baseline (speedup 1.0000x reference)
"""Multi-head self-attention (B=4, S=1024, D=1024, H=16, RoPE, causal) on 8
Trainium2 NeuronCores.

Sharding: 8 cores = 4 batches x 2 head-groups (8 heads each). Each core
computes QKV projections for its batch/head-group, RoPE, causal attention,
and a partial output projection (contraction over its 512 attention dims).
The host sums the two partial outputs per batch (the "all-reduce") and
concatenates batches.

Key implementation choices:
- All matmul operands are bfloat16 (accumulation stays f32 in PSUM). Halves
  LDWEIGHTS time, SBUF footprint and DMA traffic vs f32r.
- Q/K projection output dims are permuted to rotate-half order (evens then
  odds within each head) so RoPE works on contiguous 32-column halves.
  Permuting Q and K identically leaves Q.K^T unchanged.
- RoPE runs on the DVE in bf16 2x mode (after an ACT PSUM->bf16 copy);
  Q^T/K^T [d, s] tiles are produced by one batched DMA-engine XBAR
  transpose per (s-tile, q/k) — out[p, c, j] = in[j, 128c + p] — writing
  the [P, s-tile, pair, P] layout directly. The PE does no transposes.
- Logits are computed transposed (L^T[k, q]) so softmax sums reduce over the
  PSUM partition axis via a ones-column appended to V, and the attention
  output arrives as attn^T[c, q] which feeds the output projection directly.
- Softmax 1/sum: the [1, 512] sum row is reshaped to [128, 4] via a DRAM
  bounce so DVE reciprocal is fast (~180ns), then broadcast across 64
  partitions with a stride-0 DRAM read. The q-half-0 normalize is emitted
  as soon as EV k-tile 3 lands so only the final half is tail latency.
- Logits PSUM tiles span 2 banks so exp is one ACT instr per (head, k-tile).
"""

import numpy as np

import concourse.bass as bass
import concourse.mybir as mybir
import concourse.tile as tile
from concourse.bass import ts
from concourse.bass_utils import run_bass_kernel_spmd
from concourse.masks import make_upper_triangular

B, S, D = 4, 1024, 1024
H = 16  # total heads
HG = 8  # heads per core (head-group)
DK = 64  # head dim
DG = HG * DK  # 512, per-core projection width
PAIRS = HG // 2  # head-pair tiles of 128 rows
ROPE_THETA = 10000.0
P = 128  # partitions
NS = S // P  # 8 s-tiles
ND = D // P  # 8 d-chunks
F32 = mybir.dt.float32
BF16 = mybir.dt.bfloat16

_uid = [0]


def _split_excess_waits(nc, limit=1):
    """This container's walrus rejects >1 sync waits on the kernel-tail
    Drain; move excess waits onto same-engine NoOps inserted before it."""
    for f in nc.m.functions:
        for blk in f.blocks:
            insts = list(blk.instructions)
            out = []
            changed = False
            for inst in insts:
                si = inst.sync_info
                if si is not None and si.on_wait and len(si.on_wait) > limit:
                    waits = list(si.on_wait)
                    head, tail = waits[:-limit], waits[-limit:]
                    for i in range(0, len(head), limit):
                        _uid[0] += 1
                        nop = mybir.InstNoOp(
                            name=f"waitsplit-{_uid[0]}", ins=[], outs=[]
                        )
                        nop.engine = inst.engine
                        nop.sync_info = mybir.SyncInfo(
                            on_wait=head[i : i + limit], on_update=[]
                        )
                        out.append(nop)
                    si.on_wait = tail
                    changed = True
                out.append(inst)
            if changed:
                blk.instructions = out
    return nc


def build_nc():
    nc = bass.Bass("TRN2")
    xT = nc.dram_tensor("xT", [D, S], BF16, kind="ExternalInput")
    wqT = nc.dram_tensor("wqT", [D, DG], BF16, kind="ExternalInput")
    wkT = nc.dram_tensor("wkT", [D, DG], BF16, kind="ExternalInput")
    wvT = nc.dram_tensor("wvT", [D, DG], BF16, kind="ExternalInput")
    woT = nc.dram_tensor("woT", [DG, D], BF16, kind="ExternalInput")
    cos8 = nc.dram_tensor("cos8", [S, HG * 32], BF16, kind="ExternalInput")
    sin8 = nc.dram_tensor("sin8", [S, HG * 32], BF16, kind="ExternalInput")
    yT = nc.dram_tensor("yT", [D, S], F32, kind="ExternalOutput")
    # DRAM scratch for the softmax 1/sum reshape + broadcast bounce
    rsum = nc.dram_tensor("rsum", [HG, S], BF16)
    rrec = nc.dram_tensor("rrec", [HG, S], BF16)

    lp = nc.allow_low_precision("bf16 kernel: f32 PSUM accumulation throughout")
    lp.__enter__()
    with tile.TileContext(nc) as tc:
        with (
            tc.tile_pool(name="const", bufs=1) as constp,
            tc.tile_pool(name="wq", bufs=1) as wqp,
            tc.tile_pool(name="big", bufs=1) as bigp,
        ):
            # ztril: lower-keep causal mask for the diagonal 128x128 block
            ztril = constp.tile([P, P], BF16, tag="ztril")
            make_upper_triangular(nc, ztril[:, :], val=1.0, diag=True)

            # resident weights
            wq_all = wqp.tile([P, ND, DG], BF16, tag="wq", name="wq_all")
            wk_all = wqp.tile([P, ND, DG], BF16, tag="wk", name="wk_all")
            wv_all = wqp.tile([P, ND, DG], BF16, tag="wv", name="wv_all")
            wo_all = wqp.tile([P, DG // P, D], BF16, tag="wo", name="wo_all")
            wq_sb = [wq_all[:, c, :] for c in range(ND)]
            wk_sb = [wk_all[:, c, :] for c in range(ND)]
            wv_sb = [wv_all[:, c, :] for c in range(ND)]
            wo_sb = [wo_all[:, c, :] for c in range(DG // P)]

            def load_wo():
                nc.sync.dma_start(
                    out=wo_all[:, :, :],
                    in_=woT[:, :].rearrange("(c p) o -> p c o", p=P),
                )

            # persistent activations: Q^T/K^T in [d, pair, s] (the batched
            # XBAR transpose writes [128, 4, 128] blocks through a strided
            # 3D AP, so QK slices stay contiguous), V in [s, h, d+1] (ones
            # col for softmax sums), attn^T in [c, q]
            qt_all = bigp.tile([P, PAIRS, S], BF16, tag="qt", name="qt_all")
            kt_all = bigp.tile([P, PAIRS, S], BF16, tag="kt", name="kt_all")
            v_sb = [
                bigp.tile([P, HG, DK + 1], BF16, tag=f"v{j}", name=f"v{j}")
                for j in range(NS)
            ]
            at_sb = [
                bigp.tile([P, S], BF16, tag=f"at{p}", name=f"at{p}")
                for p in range(PAIRS)
            ]

            # ---------------- Phase A: projections + RoPE + transposes ----
            # Few, large input DMAs (the sync queue costs ~650ns per DMA
            # regardless of size): first s-tile of x, then whole weights,
            # then the rest of x and cos/sin. Nothing on the sync queue
            # waits on compute until the transposes, so loads never stall.
            with (
                tc.tile_pool(name="pa_psum", bufs=2, space="PSUM") as pap,
                tc.tile_pool(name="pa_sbuf", bufs=1) as pas,
                tc.tile_pool(name="rope", bufs=6) as ropep,
            ):
                x_all = pas.tile([P, ND, S], BF16, tag="x", name="x_all")
                xTr = xT[:, :].rearrange("(c p) s -> p c s", p=P)
                cs_all = pas.tile([P, NS, HG * 32], BF16, tag="cos")
                sn_all = pas.tile([P, NS, HG * 32], BF16, tag="sin")
                nc.sync.dma_start(out=x_all[:, :, 0:P], in_=xTr[:, :, 0:P])
                nc.sync.dma_start(
                    out=wq_all[:, :, :],
                    in_=wqT[:, :].rearrange("(c p) o -> p c o", p=P),
                )
                nc.sync.dma_start(
                    out=cs_all[:, :, :],
                    in_=cos8[:, :].rearrange("(i p) c -> p i c", p=P),
                )
                nc.sync.dma_start(
                    out=sn_all[:, :, :],
                    in_=sin8[:, :].rearrange("(i p) c -> p i c", p=P),
                )
                for w_all, wT in ((wk_all, wkT), (wv_all, wvT)):
                    nc.sync.dma_start(
                        out=w_all[:, :, :],
                        in_=wT[:, :].rearrange("(c p) o -> p c o", p=P),
                    )
                nc.sync.dma_start(
                    out=x_all[:, :, P : 4 * P], in_=xTr[:, :, P : 4 * P]
                )
                nc.sync.dma_start(
                    out=x_all[:, :, 4 * P : S], in_=xTr[:, :, 4 * P : S]
                )

                for i in range(NS):
                    xt = [x_all[:, c, ts(i, P)] for c in range(ND)]

                    qp = pap.tile([P, DG], F32, tag="q")
                    kp = pap.tile([P, DG], F32, tag="k")
                    vp = pap.tile([P, DG], F32, tag="v")
                    # sequential chains so qp/kp finish (and their ACT
                    # copies can run) before the V chain ends
                    for dst, w_sb in ((qp, wq_sb), (kp, wk_sb), (vp, wv_sb)):
                        for c in range(ND):
                            nc.tensor.matmul(
                                dst[:, :], lhsT=xt[c], rhs=w_sb[c],
                                start=(c == 0), stop=(c == ND - 1),
                            )

                    # RoPE on q/k (rotate-half layout: per head [32 ev|32 od])
                    # PSUM f32 -> bf16 SBUF copy first so the DVE ops run in
                    # 2-byte 2x mode.
                    cs3 = cs_all[:, i, :].rearrange("p (h c) -> p h c", h=HG)
                    sn3 = sn_all[:, i, :].rearrange("p (h c) -> p h c", h=HG)
                    sbq = ropep.tile([P, DG], BF16, tag="qrb")
                    nc.scalar.copy(out=sbq[:, :], in_=qp[:, :])
                    sbk = ropep.tile([P, DG], BF16, tag="krb")
                    nc.scalar.copy(out=sbk[:, :], in_=kp[:, :])
                    # V -> SBUF bf16 with a ones column per head
                    nc.scalar.copy(
                        out=v_sb[i][:, :, 0:DK],
                        in_=vp[:, :].rearrange("p (h c) -> p h c", h=HG),
                    )
                    nc.vector.memset(v_sb[i][:, :, DK : DK + 1], 1.0)
                    for sb, dst_tag in ((sbq, "qr"), (sbk, "kr")):
                        sv = sb[:, :].rearrange(
                            "p (h t c) -> p h t c", h=HG, t=2
                        )
                        ev, od = sv[:, :, 0, :], sv[:, :, 1, :]
                        r = ropep.tile([P, DG], BF16, tag=dst_tag, name=dst_tag)
                        rv = r[:, :].rearrange("p (h t c) -> p h t c", h=HG, t=2)
                        t1 = ropep.tile([P, HG * 32], BF16, tag="t1")
                        t2 = ropep.tile([P, HG * 32], BF16, tag="t2")
                        t13 = t1[:, :].rearrange("p (h c) -> p h c", h=HG)
                        t23 = t2[:, :].rearrange("p (h c) -> p h c", h=HG)
                        nc.vector.tensor_mul(t13, ev, cs3)
                        nc.vector.tensor_mul(t23, od, sn3)
                        nc.vector.tensor_sub(rv[:, :, 0, :], t13, t23)
                        t3 = ropep.tile([P, HG * 32], BF16, tag="t3")
                        t4 = ropep.tile([P, HG * 32], BF16, tag="t4")
                        t33 = t3[:, :].rearrange("p (h c) -> p h c", h=HG)
                        t43 = t4[:, :].rearrange("p (h c) -> p h c", h=HG)
                        nc.vector.tensor_mul(t33, ev, sn3)
                        nc.vector.tensor_mul(t43, od, cs3)
                        nc.vector.tensor_add(rv[:, :, 1, :], t33, t43)

                        # one batched [s, d] -> [d, s] XBAR transpose:
                        # out[p, c, j] = r[j, 128c + p] via the strided 3D
                        # out AP [[part], [S, PAIRS], [1, P]]
                        dst_all = qt_all if dst_tag == "qr" else kt_all
                        nc.sync.dma_start_transpose(
                            out=dst_all[:, :, ts(i, P)], in_=r[:, :]
                        )

            # ---------------- Phase B: attention per head ------------------
            # lg tiles span 2 PSUM banks so exp is a single ACT instr per
            # (head, k-tile). EV lags QK by 2 k-tiles. Softmax normalize for
            # q-half 0 is emitted as soon as its columns are complete
            # (after EV k-tile 3) so its latency hides behind k-tiles 4-7.
            with (
                tc.tile_pool(name="attn_psum", bufs=1, space="PSUM") as atp,
                tc.tile_pool(name="lg_psum", bufs=3, space="PSUM") as lgp,
                tc.tile_pool(name="pt_pool", bufs=6) as ptp,
                tc.tile_pool(name="sm_pool", bufs=2) as smp,
            ):
                load_wo()

                def emit_ev(h, ap, j, pt, pieces):
                    for lo, hi in pieces:
                        nc.tensor.matmul(
                            ap[:, lo:hi],
                            lhsT=v_sb[j][:, h, :],
                            rhs=pt[:, lo:hi],
                            start=(j == 0), stop=(j == NS - 1),
                            skip_group_check=True,
                        )

                def normalize(h, pair, poff, ap, hx, aps, w=512, q=None):
                    q = q or nc.sync
                    # ACT copies the finished ap columns to SBUF bf16 so the
                    # PSUM tile frees fast; then: sum row -> DRAM -> [128, c]
                    # -> reciprocal -> DRAM -> stride-0 broadcast [64, w]
                    nc.scalar.copy(
                        out=aps[:, hx : hx + w],
                        in_=ap[0 : DK + 1, hx : hx + w],
                    )
                    q.dma_start(
                        out=rsum[h, hx : hx + w].rearrange("(o c) -> o c", o=1),
                        in_=aps[DK : DK + 1, hx : hx + w],
                    )
                    rs = smp.tile([P, 4], BF16, tag="rs", name="rs")
                    q.dma_start(
                        out=rs[:, 0 : w // P],
                        in_=rsum[h, hx : hx + w].rearrange(
                            "(p c) -> p c", p=P
                        ),
                    )
                    rc = smp.tile([P, 4], BF16, tag="rc", name="rc")
                    nc.vector.reciprocal(
                        out=rc[:, 0 : w // P], in_=rs[:, 0 : w // P]
                    )
                    q.dma_start(
                        out=rrec[h, hx : hx + w].rearrange(
                            "(p c) -> p c", p=P
                        ),
                        in_=rc[:, 0 : w // P],
                    )
                    row = rrec[h, hx : hx + w]
                    bc_src = bass.AP(
                        tensor=row.tensor,
                        offset=row.offset,
                        ap=[[0, DK], [1, w]],
                    )
                    bcs = smp.tile([DK, 512], BF16, tag="bcs", name="bcs")
                    q.dma_start(out=bcs[:, 0:w], in_=bc_src)
                    if poff == 0:
                        nc.vector.tensor_mul(
                            at_sb[pair][0:DK, hx : hx + w],
                            aps[0:DK, hx : hx + w],
                            bcs[:, 0:w],
                        )
                    else:
                        tmp = smp.tile([DK, 512], BF16, tag="odd", name="odd")
                        nc.vector.tensor_mul(
                            tmp[:, 0:w], aps[0:DK, hx : hx + w], bcs[:, 0:w]
                        )
                        q.dma_start(
                            out=at_sb[pair][DK:P, hx : hx + w],
                            in_=tmp[:, 0:w],
                        )

                for h in (1, 0, 3, 2, 5, 4, 7, 6):
                    pair, poff = h // 2, DK * (h % 2)
                    last = h == 6
                    ap = atp.tile([DK + 1, S], F32, tag="attn", name=f"ap{h}")
                    aps = smp.tile(
                        [DK + 1, S], BF16, tag="aps", name=f"aps{h}"
                    )

                    def norm_after_ev(jdone):
                        # cols [0, 512) got their last EV contribution at
                        # j=3. Chains alternate between the sync and gpsimd
                        # DMA queues to halve per-queue congestion.
                        if jdone == 3:
                            normalize(
                                h, pair, poff, ap, 0, aps,
                                q=nc.gpsimd if h % 2 else nc.sync,
                            )

                    lag = 1 if last else 3
                    pending = []
                    for j in range(NS):
                        q0 = P * j
                        pieces = (
                            [(q0, 512), (512, S)] if q0 < 512 else [(q0, S)]
                        )
                        lg = lgp.tile([P, S], F32, tag="lg", name="lg")
                        for lo, hi in pieces:
                            nc.tensor.matmul(
                                lg[:, lo:hi],
                                lhsT=kt_all[poff : poff + DK, pair, ts(j, P)],
                                rhs=qt_all[poff : poff + DK, pair, lo:hi],
                                start=True, stop=True,
                            )
                        pt = ptp.tile([P, S], BF16, tag="pt", name="pt")
                        nc.scalar.activation(
                            out=pt[:, q0:S], in_=lg[:, q0:S],
                            func=mybir.ActivationFunctionType.Exp,
                            scale=0.125,
                        )
                        # zero the above-diagonal part of the diagonal block
                        nc.vector.tensor_mul(
                            pt[:, q0 : q0 + P], pt[:, q0 : q0 + P], ztril[:, :]
                        )
                        pending.append((j, pt, pieces))
                        if len(pending) > lag:
                            args = pending.pop(0)
                            emit_ev(h, ap, *args)
                            norm_after_ev(args[0])
                    for args in pending:
                        emit_ev(h, ap, *args)
                        norm_after_ev(args[0])
                    normalize(
                        h, pair, poff, ap, 512, aps,
                        q=nc.gpsimd if h % 2 else nc.sync,
                    )

            # ---------- output projection ---------------------------------
            with (
                tc.tile_pool(name="yp_psum", bufs=8, space="PSUM") as ypp,
                tc.tile_pool(name="y_sbuf", bufs=4) as ys,
            ):
                for qc in (0, 512):
                    for o in range(ND):
                        ypt = ypp.tile([P, 512], F32, tag="ypt", name="ypt")
                        for c in range(DG // P):
                            nc.tensor.matmul(
                                ypt[:, :],
                                lhsT=wo_sb[c][:, ts(o, P)],
                                rhs=at_sb[c][:, qc : qc + 512],
                                start=(c == 0), stop=(c == DG // P - 1),
                            )
                        ysb = ys.tile([P, 512], F32, tag="ysb", name="ysb")
                        nc.scalar.copy(out=ysb[:, :], in_=ypt[:, :])
                        nc.scalar.dma_start(
                            out=yT[ts(o, P), qc : qc + 512], in_=ysb[:, :]
                        )

    lp.__exit__(None, None, None)
    _split_excess_waits(nc)
    return nc


_NC_CACHE = {}


def _get_nc():
    if "nc" not in _NC_CACHE:
        _NC_CACHE["nc"] = build_nc()
    return _NC_CACHE["nc"]


# rotate-half permutation within each head: evens then odds
_PERM = np.concatenate([np.arange(0, DK, 2), np.arange(1, DK, 2)])


def _host_prep(x, Wq, Wk, Wv, Wo, token_positions):
    """Build the 8 per-core input dicts."""
    from ml_dtypes import bfloat16

    inv_freq = 1.0 / (ROPE_THETA ** (np.arange(0, DK, 2, dtype=np.float32) / DK))
    in_maps = []
    for core in range(8):
        b, g = core // 2, core % 2
        heads = np.arange(HG * g, HG * (g + 1))
        rows_qk = (heads[:, None] * DK + _PERM[None, :]).reshape(-1)
        rows_v = (heads[:, None] * DK + np.arange(DK)[None, :]).reshape(-1)
        pos = token_positions[b].astype(np.float32)  # [S]
        ang = pos[:, None] * inv_freq[None, :]  # [S, 32]
        cos8 = np.tile(np.cos(ang), (1, HG)).astype(bfloat16)
        sin8 = np.tile(np.sin(ang), (1, HG)).astype(bfloat16)
        in_maps.append(
            {
                "xT": np.ascontiguousarray(x[b].T).astype(bfloat16),
                "wqT": np.ascontiguousarray(Wq[rows_qk, :].T).astype(bfloat16),
                "wkT": np.ascontiguousarray(Wk[rows_qk, :].T).astype(bfloat16),
                "wvT": np.ascontiguousarray(Wv[rows_v, :].T).astype(bfloat16),
                "woT": np.ascontiguousarray(Wo[:, rows_v].T).astype(bfloat16),
                "cos8": cos8,
                "sin8": sin8,
            }
        )
    return in_maps


def kernel(x, Wq, Wk, Wv, Wo, token_positions, _trace=False):
    x = np.asarray(x, dtype=np.float32)
    Wq = np.asarray(Wq, dtype=np.float32)
    Wk = np.asarray(Wk, dtype=np.float32)
    Wv = np.asarray(Wv, dtype=np.float32)
    Wo = np.asarray(Wo, dtype=np.float32)
    token_positions = np.asarray(token_positions)

    nc = _get_nc()
    in_maps = _host_prep(x, Wq, Wk, Wv, Wo, token_positions)
    res = run_bass_kernel_spmd(nc, in_maps, core_ids=list(range(8)), trace=_trace)
    if _trace:
        kernel.last_exec_time_ns = res.exec_time_ns
        kernel.last_results = res

    y = np.empty((B, S, D), dtype=np.float32)
    for b in range(B):
        yT0 = res.results[2 * b]["yT"]
        yT1 = res.results[2 * b + 1]["yT"]
        y[b] = (yT0 + yT1).T
    return y
